# revision 17
# baseline (speedup 1.0000x reference)
"""Causal self-attention with RoPE for trn2, 8-core SPMD.

Problem (hardcoded): B=2, T=2048, C=1024, 16 heads, head_dim=64, fp32 io.
  qkv = x @ w_attn.T; q,k roped; causal softmax(q k^T/8) v; y @ w_proj.T

Sharding: core c -> (batch b = c//4, head-group g = c%4) — 4 heads per core.
Each core computes its group's partial output projection (bf16); host sums
the 4 group partials per batch in f32.

Device layout (per core):
  xT   [C, T]  bf16  — x[b] transposed AND pre-cast on host (DMA'd straight
                       into the SBUF-resident contraction layout)
  wqkT [C, 512] bf16 — [Wq_g | Wk_g] transposed (cols: 4 heads x 64 q, then k)
  wvT  [C, 256] bf16 — Wv_g transposed (unpadded; ones column in SBUF is
                       memset once at start)
  wpT  [256, C] bf16 — w_proj[:, group cols] transposed
  cosT/sinT [128, T] bf16 — RoPE tables transposed, 2-head stacked; sinT rows
                       0:32/64:96 pre-negated so rope = q*cos + swap32(q)*sin
  mask [128, 256] bf16 — causal 0/1 triangle (f >= p), 2-head stacked
  out  [T, C]  bf16  — partial output (host sums in f32)

vs the previous version: x is bf16 at rest (half the input DMA, no on-device
casts), RoPE reads the projection PSUM directly (no staging copy), the
block-diagonal S/exp/V work is narrowed to the causal width (cols below
d*128 of a diagonal k-tile are fully masked, so they are never computed),
the attention mask is a single 128-wide triangle multiply, and the softmax
reciprocal runs on DVE (reciprocal_approx_fast) instead of exp(-ln(s)) on
ACT — the Exp<->Ln alternation was reloading the activation table set every
pair (17 x 1.3us).  The exp table set + gpsimd library are warmed up at
t=0 under the input DMA.  Output is bf16 (half the output DMA).
"""

from contextlib import ExitStack

import numpy as np
import ml_dtypes

import concourse.bass as bass
import concourse.tile as tile
from concourse import bacc, mybir
from concourse.bass_utils import run_bass_kernel_spmd

B, T, C = 2, 2048, 1024
NH, HD = 16, 64
HG = 4              # heads per group (per core)
GD = HG * HD        # 256
NCC = C // 128      # 8 contraction chunks
F32 = mybir.dt.float32
BF16 = mybir.dt.bfloat16
BF = ml_dtypes.bfloat16

QB = 512            # query block size
KT = 128            # key tile size
ND = QB // KT       # diagonal tiles per block (4)


def build_kernel(t=T):
    nc = bacc.Bacc("TRN2", target_bir_lowering=False, debug=False)
    xT = nc.dram_tensor("xT", [C, t], BF16, kind="ExternalInput").ap()
    wqkT = nc.dram_tensor("wqkT", [C, 2 * GD], BF16, kind="ExternalInput").ap()
    wvT = nc.dram_tensor("wvT", [C, GD], BF16, kind="ExternalInput").ap()
    wpT = nc.dram_tensor("wpT", [GD, C], BF16, kind="ExternalInput").ap()
    cosT = nc.dram_tensor("cosT", [128, t], BF16, kind="ExternalInput").ap()
    sinT = nc.dram_tensor("sinT", [128, t], BF16, kind="ExternalInput").ap()
    masks = nc.dram_tensor("masks", [128, 2 * KT], BF16,
                           kind="ExternalInput").ap()
    out = nc.dram_tensor("out", [t, C], BF16, kind="ExternalOutput").ap()

    with tile.TileContext(nc) as tc:
        _attn_body(tc, out, xT, wqkT, wvT, wpT, cosT, sinT, masks, t)
    nc.compile()
    return nc


def _attn_body(tc, out, xT, wqkT, wvT, wpT, cosT, sinT, masks, t):
    ctx = ExitStack()
    nc = tc.nc
    ntt = t // 128          # t tiles (and k tiles)
    nqb = t // QB           # query blocks (== t blocks)
    Exp = mybir.ActivationFunctionType.Exp

    consts = ctx.enter_context(tc.tile_pool(name="consts", bufs=1))
    resident = ctx.enter_context(tc.tile_pool(name="resident", bufs=1))
    ropet = ctx.enter_context(tc.tile_pool(name="ropet", bufs=4))
    exps = ctx.enter_context(tc.tile_pool(name="exps", bufs=8))
    tailp = ctx.enter_context(tc.tile_pool(name="tailp", bufs=2))
    rbp = ctx.enter_context(tc.tile_pool(name="rbp", bufs=4))
    outsb = ctx.enter_context(tc.tile_pool(name="outsb", bufs=4))
    warm = ctx.enter_context(tc.tile_pool(name="warm", bufs=1))
    psA = ctx.enter_context(tc.tile_pool(name="psA", bufs=2, space="PSUM"))
    psS = ctx.enter_context(tc.tile_pool(name="psS", bufs=2, space="PSUM"))
    psY = ctx.enter_context(tc.tile_pool(name="psY", bufs=2, space="PSUM"))

    # ---- engine warmup: pull the exp table load + gpsimd library load
    # into the DMA shadow at t=0 (they cost ~2.7us / ~1us on first use)
    wsrc = warm.tile([1, 16], F32)
    nc.vector.memset(wsrc[:], 0.0)
    wexp = warm.tile([1, 16], F32)
    nc.scalar.activation(wexp[:], wsrc[:], Exp)
    wbc = warm.tile([64, 16], F32)
    nc.gpsimd.partition_broadcast(wbc[:], wexp[:])

    # ---- constants + x in (two HWDGE rings: sync carries wqk + x, the
    # scalar-engine ring carries the rest so neither blocks the other).
    # x arrives in two column waves: block-0's 512 cols first so the
    # first projection chain unblocks ~3us in, then the rest.
    x_sb = resident.tile([128, NCC, t], BF16, tag="x")
    wqk_sb = consts.tile([128, NCC, 2 * GD], BF16)
    nc.sync.dma_start(wqk_sb[:], wqkT.rearrange("(cc p) j -> p cc j", p=128))
    for cc in range(NCC):
        nc.sync.dma_start(x_sb[:, cc, 0:QB], xT[cc * 128:(cc + 1) * 128, 0:QB])
    for cc in range(NCC):
        nc.sync.dma_start(x_sb[:, cc, QB:t], xT[cc * 128:(cc + 1) * 128, QB:t])
    cos_sb = consts.tile([128, t], BF16)
    nc.scalar.dma_start(cos_sb[:], cosT[:])
    sin_sb = consts.tile([128, t], BF16)
    nc.scalar.dma_start(sin_sb[:], sinT[:])
    wv_sb = consts.tile([128, NCC, GD], BF16)
    nc.scalar.dma_start(wv_sb[:], wvT.rearrange("(cc p) j -> p cc j", p=128))
    mask_sb = consts.tile([128, 2, KT], BF16)
    nc.scalar.dma_start(mask_sb[:], masks.rearrange("p (h q) -> p h q", h=2))
    wp_sb = consts.tile([128, 2, C], BF16)
    nc.scalar.dma_start(wp_sb[:], wpT.rearrange("(jc p) c -> p jc c", p=128))

    # v tile columns: [0:64]=the head's value dims, [64]=ones (makes the
    # AV matmul also produce the exp-sums in PSUM row 64)
    VM = HD + 1        # 65
    qk = resident.tile([128, 4, t], BF16, tag="qk")
    v_sb = resident.tile([128, ntt * HG, VM], BF16, tag="v")
    nc.vector.memset(v_sb[:, :, HD:VM], 1.0)   # ones col, set once
    ynorm = resident.tile([128, 2, t], BF16, tag="ynorm")

    def qk_proj(jt, tb):
        # qk chunks: 0 = q heads(0,1), 1 = q heads(2,3), 2 = k(0,1), 3 = k(2,3)
        tsl = bass.ts(tb, QB)
        ps = psA.tile([128, QB], F32, tag="psA")
        for cc in range(NCC):
            nc.tensor.matmul(
                ps[:], wqk_sb[:, cc, bass.ts(jt, 128)], x_sb[:, cc, tsl],
                start=(cc == 0), stop=(cc == NCC - 1))
            if cc % 2 == 1:
                yield
        # rope: qk = raw*cos + swap32(raw)*sin  (sin rows pre-negated on
        # host).  The PSUM->SBUF evacuation runs on ACT (fast PSUM port,
        # identity needs no table); the muls/adds are then all-SBUF bf16
        # on DVE where they hit the packed 2x mode.
        raw = ropet.tile([128, QB], BF16, tag="raw")
        nc.scalar.copy(raw[:], ps[:])
        cosp = ropet.tile([128, QB], BF16, tag="cosp")
        nc.vector.tensor_mul(cosp[:], raw[:], cos_sb[:, tsl])
        rot = ropet.tile([128, QB], BF16, tag="rot")
        for s in range(4):
            nc.vector.tensor_copy(rot[s * 32:(s + 1) * 32, :],
                                  raw[(s ^ 1) * 32:((s ^ 1) + 1) * 32, :])
        sinp = ropet.tile([128, QB], BF16, tag="sinp")
        nc.vector.tensor_mul(sinp[:], rot[:], sin_sb[:, tsl])
        yield
        nc.vector.tensor_add(qk[:, jt, tsl], cosp[:], sinp[:])
        yield

    def v_proj(tt):
        # v layout [128, ntt*HG, 65]: (t-tile, local head): 64 cols + ones
        ps = psA.tile([128, GD], F32, tag="psA")
        for cc in range(NCC):
            nc.tensor.matmul(
                ps[:], x_sb[:, cc, bass.ts(tt, 128)], wv_sb[:, cc, :],
                start=(cc == 0), stop=(cc == NCC - 1))
            if cc % 2 == 1:
                yield
        nc.vector.tensor_copy(
            v_sb[:, tt * HG:(tt + 1) * HG, 0:HD],
            ps.rearrange("p (h d) -> p h d", d=HD))
        yield

    def attention_steps(qb, p):
        """Generator yielding one kt-step at a time (software-pipelined:
        S^T/exp for kt runs one step ahead of the V matmuls).  Work on the
        block-diagonal k-tiles is narrowed to the causal width: tile kt of
        block qb only touches q-cols >= d*KT (d = kt - qb*ND); everything
        left of that is fully masked and never computed or read."""
        qsl = bass.ts(qb, QB)
        nkt = (qb + 1) * ND
        qc = qk[:, p, :]
        kc = qk[:, 2 + p, :]
        ya = psY.tile([VM, QB], F32, tag="psY")
        yb = psY.tile([VM, QB], F32, tag="psY")
        ets = {}
        for kt in range(nkt + 1):
            # V matmuls for the previous kt go first: they are ready (their
            # exp finished a step ago) while S(kt) may still wait on a PSUM
            # slot — keep the PE FIFO unblocked
            if kt >= 1:
                kv = kt - 1
                et, q0 = ets.pop(kv)
                first, last = (kv == 0), (kv == nkt - 1)
                nc.tensor.matmul(ya[:, q0:QB], v_sb[:, kv * HG + 2 * p, :],
                                 et[:, 0, q0:QB], start=first, stop=last)
                nc.tensor.matmul(yb[:, q0:QB], v_sb[:, kv * HG + 2 * p + 1, :],
                                 et[:, 1, q0:QB], start=first, stop=last)
                yield
            if kt < nkt:
                d = kt - qb * ND
                q0 = max(d, 0) * KT
                ksl = bass.ts(kt, KT)
                qn = qc[0:64, qb * QB + q0:(qb + 1) * QB]
                kn = kc[0:64, ksl]
                pss = psS.tile([128, 2, QB], F32, tag="psS")
                nc.tensor.matmul(pss[:, 0, q0:QB], kn, qn,
                                 start=True, stop=True, tile_position=(0, 0))
                nc.tensor.matmul(pss[:, 1, q0:QB], kc[64:128, ksl],
                                 qc[64:128, qb * QB + q0:(qb + 1) * QB],
                                 start=True, stop=True, tile_position=(64, 0))
                et = exps.tile([128, 2, QB], BF16, tag="exps")
                nc.scalar.activation(et[:, :, q0:QB], pss[:, :, q0:QB],
                                     Exp, scale=0.125)
                if d >= 0:  # block-diagonal: 128-wide causal triangle mask
                    # on gpsimd — it's otherwise idle and this keeps the
                    # exp->mask->AV chain off the busy DVE queue
                    nc.gpsimd.tensor_mul(et[:, :, q0:q0 + KT],
                                         et[:, :, q0:q0 + KT], mask_sb[:])
                ets[kt] = (et, q0)
            yield
        # softmax denominators: the ones column of v makes ya/yb row 64 the
        # exp-sums.  ACT (fast PSUM port, identity fn, no table load)
        # stages that row to a partition-0 tile for partition_broadcast —
        # which only reads physical partition 0 — while DVE moves y to
        # SBUF (frees psY).  1/s runs on DVE (reciprocal_approx_fast — no
        # Ln, no ACT table switch).
        srow = rbp.tile([1, 2 * QB], F32, tag="srow")
        nc.scalar.copy(srow[:, 0:QB], ya[HD:VM, :])
        nc.scalar.copy(srow[:, QB:2 * QB], yb[HD:VM, :])
        yab = tailp.tile([HD, 2 * QB], F32, tag="yab")
        nc.vector.tensor_copy(yab[:, 0:QB], ya[0:HD, :])
        nc.vector.tensor_copy(yab[:, QB:2 * QB], yb[0:HD, :])
        yield
        rb = rbp.tile([HD, 2 * QB], F32, tag="rb")
        nc.gpsimd.partition_broadcast(rb[:], srow[:])
        rc = rbp.tile([HD, 2 * QB], F32, tag="rc")
        nc.vector.reciprocal_approx_fast(rc[:], rb[:])
        yield
        nc.vector.tensor_mul(ynorm[0:HD, p, qsl], yab[:, 0:QB],
                             rc[:, 0:QB])
        nc.vector.tensor_mul(ynorm[HD:2 * HD, p, qsl], yab[:, QB:2 * QB],
                             rc[:, QB:2 * QB])
        yield

    def out_proj(tt, cb):
        ps = psA.tile([128, QB], F32, tag="psA")
        nc.tensor.matmul(ps[:], ynorm[:, 0, bass.ts(tt, 128)],
                         wp_sb[:, 0, bass.ts(cb, QB)], start=True, stop=False)
        yield
        nc.tensor.matmul(ps[:], ynorm[:, 1, bass.ts(tt, 128)],
                         wp_sb[:, 1, bass.ts(cb, QB)], start=False, stop=True)
        ot = outsb.tile([128, QB], BF16, tag="ot")
        nc.vector.tensor_copy(ot[:], ps[:])
        nc.sync.dma_start(
            out[tt * 128:(tt + 1) * 128, bass.ts(cb, QB)], ot[:])
        yield

    # ---- interleaved schedule ----
    # Projection for block tb feeds attention for qb=tb (causal attention
    # needs K/V only up to the diagonal).  Attention's kt-steps are woven
    # with small granules of the *next* block's projection and the
    # *previous* block's output projection, so the PE always has a little
    # independent matmul work queued while ACT chews through exp, without
    # long FIFO chains delaying the next S^T matmul.
    from collections import deque
    proj_fill = deque()   # next block's qk/v projection granules
    out_fill = deque()    # completed blocks' output-projection granules

    def drain(n):
        for _ in range(n):
            q = proj_fill if proj_fill else out_fill
            if not q:
                return
            g = q.popleft()
            try:
                next(g)
                q.append(g)
            except StopIteration:
                pass

    def force(q):
        while q:
            g = q.popleft()
            for _ in g:
                pass

    for jt in (0, 2, 1, 3):
        for _ in qk_proj(jt, 0):
            pass
    for tt in range(4):
        for _ in v_proj(tt):
            pass
    for tb in range(nqb):
        force(proj_fill)  # attention(tb) needs block tb's projections traced
        if tb + 1 < nqb:
            for jt in range(4):
                proj_fill.append(qk_proj(jt, tb + 1))
            for tt in range((tb + 1) * 4, (tb + 1) * 4 + 4):
                proj_fill.append(v_proj(tt))
        for p in range(2):
            for _ in attention_steps(tb, p):
                drain(2 if len(proj_fill) > 5 else 1)
        for tt in range(tb * 4, tb * 4 + 4):
            for cb in range(2):
                out_fill.append(out_proj(tt, cb))
    force(proj_fill)
    force(out_fill)
    ctx.close()


def host_inputs(x, w_attn, w_proj, t=T):
    """Build the 8 per-core input maps from full inputs."""
    xTs = [np.ascontiguousarray(x[b, :t].T).astype(BF)
           for b in range(B)]
    inv = 1.0 / (10000.0 ** (np.arange(0, HD, 2, dtype=np.float32) / HD))
    fr = np.outer(np.arange(t, dtype=np.float32), inv)     # [t, 32]
    emb = np.concatenate([fr, fr], 1)                      # [t, 64]
    cos = np.cos(emb).T.astype(np.float32)                 # [64, t]
    sin = np.sin(emb).T.astype(np.float32)
    sin_s = sin.copy()
    sin_s[:32] *= -1.0
    cosT2 = np.tile(cos, (2, 1)).astype(BF)
    sinT2 = np.tile(sin_s, (2, 1)).astype(BF)

    # causal triangle for a block-diagonal k-tile: keep iff f >= p, for
    # f in [0,KT), p in [0,KT), doubled over the head pair
    f = np.arange(KT)[None, :]
    pp = np.arange(KT)[:, None]
    m = (f >= pp)
    masks = np.concatenate([m, m], axis=1).astype(BF)      # [128, 256]

    in_maps = []
    for c in range(8):
        b, g = c // 4, c % 4
        wq = w_attn[g * GD:(g + 1) * GD]
        wk = w_attn[C + g * GD:C + (g + 1) * GD]
        wv = w_attn[2 * C + g * GD:2 * C + (g + 1) * GD]
        wqkT = np.ascontiguousarray(
            np.concatenate([wq, wk], 0).T).astype(BF)
        wvT = np.ascontiguousarray(wv.T).astype(BF)
        wpT = np.ascontiguousarray(
            w_proj[:, g * GD:(g + 1) * GD].T).astype(BF)
        in_maps.append({"xT": xTs[b], "wqkT": wqkT, "wvT": wvT,
                        "wpT": wpT, "cosT": cosT2, "sinT": sinT2,
                        "masks": masks})
    return in_maps


_cache = {}


def kernel(x, w_attn, w_proj):
    x = np.asarray(x, dtype=np.float32)
    w_attn = np.asarray(w_attn, dtype=np.float32)
    w_proj = np.asarray(w_proj, dtype=np.float32)
    if "nc" not in _cache:
        _cache["nc"] = build_kernel()
    nc = _cache["nc"]
    in_maps = host_inputs(x, w_attn, w_proj)
    res = run_bass_kernel_spmd(nc, in_maps, list(range(8)))
    out = np.zeros((B, T, C), dtype=np.float32)
    for c in range(8):
        out[c // 4] += res.results[c]["out"].astype(np.float32)
    return out


# revision 20
# speedup vs baseline: 1.0319x; 1.0319x over previous
"""Causal self-attention with RoPE for trn2, 8-core SPMD.

Problem (hardcoded): B=2, T=2048, C=1024, 16 heads, head_dim=64, fp32 io.
  qkv = x @ w_attn.T; q,k roped; causal softmax(q k^T/8) v; y @ w_proj.T

Sharding: core c -> (batch b = c//4, head-group g = c%4) — 4 heads per core.
Each core computes its group's partial output projection (bf16); host sums
the 4 group partials per batch in f32.

Device layout (per core):
  xT   [C, T]  bf16  — x[b] transposed AND pre-cast on host (DMA'd straight
                       into the SBUF-resident contraction layout)
  wqkT [C, 512] bf16 — [Wq_g | Wk_g] transposed (cols: 4 heads x 64 q, then k)
  wvT  [C, 256] bf16 — Wv_g transposed (unpadded; ones column in SBUF is
                       memset once at start)
  wpT  [256, C] bf16 — w_proj[:, group cols] transposed
  cosT/sinT [128, T] bf16 — RoPE tables transposed, 2-head stacked; sinT rows
                       0:32/64:96 pre-negated so rope = q*cos + swap32(q)*sin
  mask [128, 256] bf16 — causal 0/1 triangle (f >= p), 2-head stacked
  out  [T, C]  bf16  — partial output (host sums in f32)

vs the previous version: x is bf16 at rest (half the input DMA, no on-device
casts), RoPE reads the projection PSUM directly (no staging copy), the
block-diagonal S/exp/V work is narrowed to the causal width (cols below
d*128 of a diagonal k-tile are fully masked, so they are never computed),
the attention mask is a single 128-wide triangle multiply, and the softmax
reciprocal runs on DVE (reciprocal_approx_fast) instead of exp(-ln(s)) on
ACT — the Exp<->Ln alternation was reloading the activation table set every
pair (17 x 1.3us).  The exp table set + gpsimd library are warmed up at
t=0 under the input DMA.  Output is bf16 (half the output DMA).
"""

from contextlib import ExitStack

import numpy as np
import ml_dtypes

import concourse.bass as bass
import concourse.tile as tile
from concourse import bacc, mybir
from concourse.bass_utils import run_bass_kernel_spmd

B, T, C = 2, 2048, 1024
NH, HD = 16, 64
HG = 4              # heads per group (per core)
GD = HG * HD        # 256
NCC = C // 128      # 8 contraction chunks
F32 = mybir.dt.float32
BF16 = mybir.dt.bfloat16
BF = ml_dtypes.bfloat16

QB = 512            # query block size
KT = 128            # key tile size
ND = QB // KT       # diagonal tiles per block (4)


def build_kernel(t=T):
    nc = bacc.Bacc("TRN2", target_bir_lowering=False, debug=False)
    xT = nc.dram_tensor("xT", [C, t], BF16, kind="ExternalInput").ap()
    wqkT = nc.dram_tensor("wqkT", [C, 2 * GD], BF16, kind="ExternalInput").ap()
    wvT = nc.dram_tensor("wvT", [C, GD], BF16, kind="ExternalInput").ap()
    wpT = nc.dram_tensor("wpT", [GD, C], BF16, kind="ExternalInput").ap()
    cosT = nc.dram_tensor("cosT", [128, t], BF16, kind="ExternalInput").ap()
    sinT = nc.dram_tensor("sinT", [128, t], BF16, kind="ExternalInput").ap()
    masks = nc.dram_tensor("masks", [128, 2 * KT], BF16,
                           kind="ExternalInput").ap()
    out = nc.dram_tensor("out", [t, C], BF16, kind="ExternalOutput").ap()

    with tile.TileContext(nc) as tc:
        _attn_body(tc, out, xT, wqkT, wvT, wpT, cosT, sinT, masks, t)
    nc.compile()
    return nc


def _attn_body(tc, out, xT, wqkT, wvT, wpT, cosT, sinT, masks, t):
    ctx = ExitStack()
    nc = tc.nc
    ntt = t // 128          # t tiles (and k tiles)
    nqb = t // QB           # query blocks (== t blocks)
    Exp = mybir.ActivationFunctionType.Exp

    consts = ctx.enter_context(tc.tile_pool(name="consts", bufs=1))
    resident = ctx.enter_context(tc.tile_pool(name="resident", bufs=1))
    ropet = ctx.enter_context(tc.tile_pool(name="ropet", bufs=4))
    exps = ctx.enter_context(tc.tile_pool(name="exps", bufs=8))
    tailp = ctx.enter_context(tc.tile_pool(name="tailp", bufs=2))
    rbp = ctx.enter_context(tc.tile_pool(name="rbp", bufs=4))
    outsb = ctx.enter_context(tc.tile_pool(name="outsb", bufs=4))
    warm = ctx.enter_context(tc.tile_pool(name="warm", bufs=1))
    psA = ctx.enter_context(tc.tile_pool(name="psA", bufs=2, space="PSUM"))
    psS = ctx.enter_context(tc.tile_pool(name="psS", bufs=2, space="PSUM"))
    psY = ctx.enter_context(tc.tile_pool(name="psY", bufs=2, space="PSUM"))

    # ---- engine warmup: pull the exp table load + gpsimd library load
    # into the DMA shadow at t=0 (they cost ~2.7us / ~1us on first use)
    wsrc = warm.tile([1, 16], F32)
    nc.vector.memset(wsrc[:], 0.0)
    wexp = warm.tile([1, 16], F32)
    nc.scalar.activation(wexp[:], wsrc[:], Exp)
    wbc = warm.tile([64, 16], F32)
    nc.gpsimd.partition_broadcast(wbc[:], wexp[:])

    # ---- constants + x in, all on the sync HWDGE ring, ordered by when
    # each consumer first needs the data: wqk + block-0's x columns feed
    # the first projection chains (~6us), wv/cos/sin/mask unblock the
    # first v-proj / rope / diagonal exp (~7-10us), then the rest of x
    # (needed by block-1 projection ~18us) and wp (first out-proj ~25us).
    x_sb = resident.tile([128, NCC, t], BF16, tag="x")
    wqk_sb = consts.tile([128, NCC, 2 * GD], BF16)
    nc.sync.dma_start(wqk_sb[:], wqkT.rearrange("(cc p) j -> p cc j", p=128))
    for cc in range(NCC):
        nc.sync.dma_start(x_sb[:, cc, 0:QB], xT[cc * 128:(cc + 1) * 128, 0:QB])
    wv_sb = consts.tile([128, NCC, GD], BF16)
    nc.sync.dma_start(wv_sb[:], wvT.rearrange("(cc p) j -> p cc j", p=128))
    cos_sb = consts.tile([128, t], BF16)
    nc.sync.dma_start(cos_sb[:], cosT[:])
    sin_sb = consts.tile([128, t], BF16)
    nc.sync.dma_start(sin_sb[:], sinT[:])
    mask_sb = consts.tile([128, 2, KT], BF16)
    nc.sync.dma_start(mask_sb[:], masks.rearrange("p (h q) -> p h q", h=2))
    for cc in range(NCC):
        nc.sync.dma_start(x_sb[:, cc, QB:t], xT[cc * 128:(cc + 1) * 128, QB:t])
    wp_sb = consts.tile([128, 2, C], BF16)
    nc.sync.dma_start(wp_sb[:], wpT.rearrange("(jc p) c -> p jc c", p=128))

    # v tile columns: [0:64]=the head's value dims, [64]=ones (makes the
    # AV matmul also produce the exp-sums in PSUM row 64)
    VM = HD + 1        # 65
    qk = resident.tile([128, 4, t], BF16, tag="qk")
    v_sb = resident.tile([128, ntt * HG, VM], BF16, tag="v")
    nc.vector.memset(v_sb[:, :, HD:VM], 1.0)   # ones col, set once
    ynorm = resident.tile([128, 2, t], BF16, tag="ynorm")

    def qk_proj(jt, tb):
        # qk chunks: 0 = q heads(0,1), 1 = q heads(2,3), 2 = k(0,1), 3 = k(2,3)
        tsl = bass.ts(tb, QB)
        ps = psA.tile([128, QB], F32, tag="psA")
        for cc in range(NCC):
            nc.tensor.matmul(
                ps[:], wqk_sb[:, cc, bass.ts(jt, 128)], x_sb[:, cc, tsl],
                start=(cc == 0), stop=(cc == NCC - 1))
            if cc % 2 == 1:
                yield
        # rope: qk = raw*cos + swap32(raw)*sin  (sin rows pre-negated on
        # host).  The PSUM->SBUF evacuation runs on ACT (fast PSUM port,
        # identity needs no table); the muls/adds are then all-SBUF bf16
        # on DVE where they hit the packed 2x mode.
        raw = ropet.tile([128, QB], BF16, tag="raw")
        nc.scalar.copy(raw[:], ps[:])
        cosp = ropet.tile([128, QB], BF16, tag="cosp")
        nc.vector.tensor_mul(cosp[:], raw[:], cos_sb[:, tsl])
        rot = ropet.tile([128, QB], BF16, tag="rot")
        for s in range(4):
            nc.vector.tensor_copy(rot[s * 32:(s + 1) * 32, :],
                                  raw[(s ^ 1) * 32:((s ^ 1) + 1) * 32, :])
        sinp = ropet.tile([128, QB], BF16, tag="sinp")
        nc.vector.tensor_mul(sinp[:], rot[:], sin_sb[:, tsl])
        yield
        nc.vector.tensor_add(qk[:, jt, tsl], cosp[:], sinp[:])
        yield

    def v_proj(tt):
        # v layout [128, ntt*HG, 65]: (t-tile, local head): 64 cols + ones
        ps = psA.tile([128, GD], F32, tag="psA")
        for cc in range(NCC):
            nc.tensor.matmul(
                ps[:], x_sb[:, cc, bass.ts(tt, 128)], wv_sb[:, cc, :],
                start=(cc == 0), stop=(cc == NCC - 1))
            if cc % 2 == 1:
                yield
        nc.vector.tensor_copy(
            v_sb[:, tt * HG:(tt + 1) * HG, 0:HD],
            ps.rearrange("p (h d) -> p h d", d=HD))
        yield

    def attention_steps(qb, p):
        """Generator yielding one kt-step at a time (software-pipelined:
        S^T/exp for kt runs one step ahead of the V matmuls).  Work on the
        block-diagonal k-tiles is narrowed to the causal width: tile kt of
        block qb only touches q-cols >= d*KT (d = kt - qb*ND); everything
        left of that is fully masked and never computed or read."""
        qsl = bass.ts(qb, QB)
        nkt = (qb + 1) * ND
        qc = qk[:, p, :]
        kc = qk[:, 2 + p, :]
        ya = psY.tile([VM, QB], F32, tag="psY")
        yb = psY.tile([VM, QB], F32, tag="psY")
        ets = {}
        for kt in range(nkt + 1):
            # V matmuls for the previous kt go first: they are ready (their
            # exp finished a step ago) while S(kt) may still wait on a PSUM
            # slot — keep the PE FIFO unblocked
            if kt >= 1:
                kv = kt - 1
                et, q0 = ets.pop(kv)
                first, last = (kv == 0), (kv == nkt - 1)
                nc.tensor.matmul(ya[:, q0:QB], v_sb[:, kv * HG + 2 * p, :],
                                 et[:, 0, q0:QB], start=first, stop=last)
                nc.tensor.matmul(yb[:, q0:QB], v_sb[:, kv * HG + 2 * p + 1, :],
                                 et[:, 1, q0:QB], start=first, stop=last)
                yield
            if kt < nkt:
                d = kt - qb * ND
                q0 = max(d, 0) * KT
                ksl = bass.ts(kt, KT)
                qn = qc[0:64, qb * QB + q0:(qb + 1) * QB]
                kn = kc[0:64, ksl]
                pss = psS.tile([128, 2, QB], F32, tag="psS")
                nc.tensor.matmul(pss[:, 0, q0:QB], kn, qn,
                                 start=True, stop=True, tile_position=(0, 0))
                nc.tensor.matmul(pss[:, 1, q0:QB], kc[64:128, ksl],
                                 qc[64:128, qb * QB + q0:(qb + 1) * QB],
                                 start=True, stop=True, tile_position=(64, 0))
                et = exps.tile([128, 2, QB], BF16, tag="exps")
                nc.scalar.activation(et[:, :, q0:QB], pss[:, :, q0:QB],
                                     Exp, scale=0.125)
                if d >= 0:  # block-diagonal: 128-wide causal triangle mask
                    # on gpsimd — it's otherwise idle and this keeps the
                    # exp->mask->AV chain off the busy DVE queue
                    nc.gpsimd.tensor_mul(et[:, :, q0:q0 + KT],
                                         et[:, :, q0:q0 + KT], mask_sb[:])
                ets[kt] = (et, q0)
            yield
        # softmax denominators: the ones column of v makes ya/yb row 64 the
        # exp-sums.  ACT (fast PSUM port, identity fn, no table load) is
        # the SOLE psY reader so the banks free ~1.4us after the last V
        # matmul regardless of DVE queue depth; it then stages the sums
        # row to a partition-0 tile for partition_broadcast — which only
        # reads physical partition 0.  1/s runs on DVE
        # (reciprocal_approx_fast — no Ln, no ACT table switch).
        yab = tailp.tile([VM, 2 * QB], F32, tag="yab")
        nc.scalar.copy(yab[:, 0:QB], ya[:])
        nc.scalar.copy(yab[:, QB:2 * QB], yb[:])
        srow = rbp.tile([1, 2 * QB], F32, tag="srow")
        nc.scalar.copy(srow[:], yab[HD:VM, :])
        yield
        rb = rbp.tile([HD, 2 * QB], F32, tag="rb")
        nc.gpsimd.partition_broadcast(rb[:], srow[:])
        rc = rbp.tile([HD, 2 * QB], F32, tag="rc")
        nc.vector.reciprocal_approx_fast(rc[:], rb[:])
        yield
        nc.vector.tensor_mul(ynorm[0:HD, p, qsl], yab[0:HD, 0:QB],
                             rc[:, 0:QB])
        nc.vector.tensor_mul(ynorm[HD:2 * HD, p, qsl], yab[0:HD, QB:2 * QB],
                             rc[:, QB:2 * QB])
        yield

    def out_proj(tt, cb):
        ps = psA.tile([128, QB], F32, tag="psA")
        nc.tensor.matmul(ps[:], ynorm[:, 0, bass.ts(tt, 128)],
                         wp_sb[:, 0, bass.ts(cb, QB)], start=True, stop=False)
        yield
        nc.tensor.matmul(ps[:], ynorm[:, 1, bass.ts(tt, 128)],
                         wp_sb[:, 1, bass.ts(cb, QB)], start=False, stop=True)
        ot = outsb.tile([128, QB], BF16, tag="ot")
        nc.vector.tensor_copy(ot[:], ps[:])
        nc.sync.dma_start(
            out[tt * 128:(tt + 1) * 128, bass.ts(cb, QB)], ot[:])
        yield

    # ---- interleaved schedule ----
    # Projection for block tb feeds attention for qb=tb (causal attention
    # needs K/V only up to the diagonal).  Attention's kt-steps are woven
    # with small granules of the *next* block's projection and the
    # *previous* block's output projection, so the PE always has a little
    # independent matmul work queued while ACT chews through exp, without
    # long FIFO chains delaying the next S^T matmul.
    from collections import deque
    proj_fill = deque()   # next block's qk/v projection granules
    out_fill = deque()    # completed blocks' output-projection granules

    def drain(n):
        for _ in range(n):
            q = proj_fill if proj_fill else out_fill
            if not q:
                return
            g = q.popleft()
            try:
                next(g)
                q.append(g)
            except StopIteration:
                pass

    def force(q):
        while q:
            g = q.popleft()
            for _ in g:
                pass

    for jt in (0, 2, 1, 3):
        for _ in qk_proj(jt, 0):
            pass
    for tt in range(4):
        for _ in v_proj(tt):
            pass
    for tb in range(nqb):
        force(proj_fill)  # attention(tb) needs block tb's projections traced
        if tb + 1 < nqb:
            for jt in range(4):
                proj_fill.append(qk_proj(jt, tb + 1))
            for tt in range((tb + 1) * 4, (tb + 1) * 4 + 4):
                proj_fill.append(v_proj(tt))
        for p in range(2):
            for _ in attention_steps(tb, p):
                drain(2 if len(proj_fill) > 5 else 1)
        for tt in range(tb * 4, tb * 4 + 4):
            for cb in range(2):
                out_fill.append(out_proj(tt, cb))
    force(proj_fill)
    force(out_fill)
    ctx.close()


def host_inputs(x, w_attn, w_proj, t=T):
    """Build the 8 per-core input maps from full inputs."""
    xTs = [np.ascontiguousarray(x[b, :t].T).astype(BF)
           for b in range(B)]
    inv = 1.0 / (10000.0 ** (np.arange(0, HD, 2, dtype=np.float32) / HD))
    fr = np.outer(np.arange(t, dtype=np.float32), inv)     # [t, 32]
    emb = np.concatenate([fr, fr], 1)                      # [t, 64]
    cos = np.cos(emb).T.astype(np.float32)                 # [64, t]
    sin = np.sin(emb).T.astype(np.float32)
    sin_s = sin.copy()
    sin_s[:32] *= -1.0
    cosT2 = np.tile(cos, (2, 1)).astype(BF)
    sinT2 = np.tile(sin_s, (2, 1)).astype(BF)

    # causal triangle for a block-diagonal k-tile: keep iff f >= p, for
    # f in [0,KT), p in [0,KT), doubled over the head pair
    f = np.arange(KT)[None, :]
    pp = np.arange(KT)[:, None]
    m = (f >= pp)
    masks = np.concatenate([m, m], axis=1).astype(BF)      # [128, 256]

    in_maps = []
    for c in range(8):
        b, g = c // 4, c % 4
        wq = w_attn[g * GD:(g + 1) * GD]
        wk = w_attn[C + g * GD:C + (g + 1) * GD]
        wv = w_attn[2 * C + g * GD:2 * C + (g + 1) * GD]
        wqkT = np.ascontiguousarray(
            np.concatenate([wq, wk], 0).T).astype(BF)
        wvT = np.ascontiguousarray(wv.T).astype(BF)
        wpT = np.ascontiguousarray(
            w_proj[:, g * GD:(g + 1) * GD].T).astype(BF)
        in_maps.append({"xT": xTs[b], "wqkT": wqkT, "wvT": wvT,
                        "wpT": wpT, "cosT": cosT2, "sinT": sinT2,
                        "masks": masks})
    return in_maps


_cache = {}


def kernel(x, w_attn, w_proj):
    x = np.asarray(x, dtype=np.float32)
    w_attn = np.asarray(w_attn, dtype=np.float32)
    w_proj = np.asarray(w_proj, dtype=np.float32)
    if "nc" not in _cache:
        _cache["nc"] = build_kernel()
    nc = _cache["nc"]
    in_maps = host_inputs(x, w_attn, w_proj)
    res = run_bass_kernel_spmd(nc, in_maps, list(range(8)))
    out = np.zeros((B, T, C), dtype=np.float32)
    for c in range(8):
        out[c // 4] += res.results[c]["out"].astype(np.float32)
    return out
